# revision 13
# baseline (speedup 1.0000x reference)
"""LinearSpline activation kernel for Trainium2 (8 NeuronCores, SPMD).

Math: per channel c the reference computes a 51-knot uniform linear spline,
which as a function of t' = x*scale/grid + 25 is a 50-segment piecewise-linear
function with breakpoints at integers 1..49 and linear extrapolation outside.
We evaluate it in the slope-change (relu) basis
    g(t') = A + b*t' + sum_{i=1..49} gamma_i * max(t' - i, 0)   (+ consts in A)
(all constants folded per channel on the host, incl. the 1/scale output
factor).

Engine split per [128, 2048] tile:
  ACT   : t' stream, the affine seed (b*t' + A), and 9 shifted streams
          v_g = t' - off_g for the DVE partial ops
  DVE   : 9 unchained 3-term partial ops (terms i=23..49) on the ACT
          shifted streams, then 11 chained 2-term ops (terms i=1..22; the
          shift rides imm2 so they all read the same t' stream; partial
          p_0 seeds the chain)
  GPSIMD: tensor_tensor merges: seed + partials p_1..p_8 + DVE chain -> out
Layout: data-parallel over batch (4 per core), partition p = n2*64 + channel,
so per-channel constants ride [P,1] scalar operands.
"""

import numpy as np

# ---------------- problem constants (hardcoded; kernel.py is standalone) ---
N_BATCH, N_CH, H, W = 32, 64, 128, 128
NCORES = 8
SIZE = 51
GRID = np.float64(2.0 * 4.0 / (SIZE - 1))  # 0.16
F = 2048                   # free-dim chunk per unit
GROUPS = 2                 # batch pairs per core
FREE = H * W               # 16384 free elems per (group, partition)
UNITS = GROUPS * (FREE // F)   # 16
N_CHAIN = 11               # chain ops: seed(i=1,2) + 10 pairs (i=3..22)
P_OFFS = [23 + 3 * k for k in range(9)]   # partial-op base knots (23..47)
N_PART = len(P_OFFS)       # 9 partial ops, terms i=23..49
N_DVE = N_CHAIN + N_PART   # 20
N_ACT = 2 + N_PART         # t' + seed + 9 shifted streams
N_GP = N_PART              # 8 partial merges + final merge (p_0 seeds the DVE chain)
RING_V = 5                 # shifted-stream ring depth
RING_P = 6                 # partial-buffer ring depth

_f32, _f64 = np.float32, np.float64
_built = {}


def _host_params(coeff, scal):
    """coeff [3264], scal [64] -> per-channel table [64, 63] f32.
    cols: 0 a_s, 1 A, 2 b, 3..51 gamma_1..gamma_49, 52 t'-bias (25),
    53..61 partial-stream biases 25-off."""
    C = coeff.reshape(N_CH, SIZE).astype(_f64)
    d = C[:, 1:] - C[:, :-1]               # [64, 50]
    s = scal.astype(_f64)
    b = d[:, 0] / s
    gam = (d[:, 1:] - d[:, :-1]) / s[:, None]   # [64, 49]; gamma_i at col i-1
    # calibrate A at t'=25 (t=0) against the exact device-side term forms:
    #   chain terms (i=1..22):  gamma_i * max(25, i)
    #   partial terms (i>=23, base off): gamma_i * max(25 - off, i - off)
    g0 = b * 25.0
    for i in range(1, 23):
        g0 = g0 + gam[:, i - 1] * max(25.0, float(i))
    for off in P_OFFS:
        for k in range(3):
            i = off + k
            g0 = g0 + gam[:, i - 1] * max(25.0 - off, float(k))
    A = C[:, 25] / s - g0
    prm = np.zeros((N_CH, 64), _f64)
    prm[:, 0] = s / GRID
    prm[:, 1] = A
    prm[:, 2] = b
    prm[:, 3:52] = gam
    prm[:, 52] = 25.0
    for k, off in enumerate(P_OFFS):
        prm[:, 53 + k] = 25.0 - off
    prm[:, 62] = b * prm[:, 0]            # seed scale: b*a_s
    prm[:, 63] = 25.0 * b + A             # seed bias
    return prm.astype(_f32)


def _register_ops():
    import concourse.dve_ops as dve_ops
    from concourse.dve_spec import (
        Spec, Src0, Src1, C0, C1, C2, C3, Zero, One, lower, maxx,
        _spill_c3_to_src1,
    )
    from concourse.dve_uop import DveOpSpec

    def reg(name, spec, rd1):
        for op in dve_ops.OPS:
            if op.name == name:
                return op
        row = max(dve_ops._SUB_OPCODE_FOR_NAME.values()) + 1
        assert row < 0x20
        dve_ops._SUB_OPCODE_FOR_NAME[name] = row
        uops = lower(spec, ver="v3")
        sha = DveOpSpec(name=name, opcode=row, uops=uops, rd1_en=rd1).sha("v3")
        op = dve_ops.DveOp(name, spec, subdim=False, uops_sha={"v3": sha})
        dve_ops.OPS.append(op)
        dve_ops.CUSTOM_DVE_SPECS[name] = spec
        return op

    TWO = One + One
    # seed:  C0*max(t', j) + C1*max(t', j+1), j = imm2
    seed = reg("LS_R2SEED", Spec(
        body=C0 * maxx(Src0, C2) + C1 * maxx(Src0, C2 + One)), rd1=False)
    # chain: acc + C0*max(t', j) + C1*max(t', j+1), j = imm2
    chain = reg("LS_R2CHAIN", Spec(
        body=Src1 + C0 * maxx(Src0, C2) + C1 * maxx(Src0, C2 + One)), rd1=True)
    # partial: C0*max(v,0) + C1*max(v,1) + C3*max(v,2), v = t' - off
    # (third weight rides C3, spilled to a latched read of Src1)
    part = reg("LS_R3PART", Spec(
        body=_spill_c3_to_src1(
            C0 * maxx(Src0, Zero) + C1 * maxx(Src0, One)
            + C3 * maxx(Src0, TWO))), rd1=True)
    return seed, chain, part


def _build():
    if "nc" in _built:
        return _built["nc"]
    import concourse.bass as bass
    import concourse.mybir as mybir
    from concourse.library_overlay import lower_extended_insts

    SEED, CHAIN, PART = _register_ops()
    F32 = mybir.dt.float32
    Ident = mybir.ActivationFunctionType.Identity
    Alu = mybir.AluOpType

    nc = bass.Bass()
    x_in = nc.declare_dram_parameter("x", [GROUPS, 128, FREE], F32, isOutput=False)
    prm = nc.declare_dram_parameter("prm", [128, 64], F32, isOutput=False)
    y_out = nc.declare_dram_parameter("y", [GROUPS, 128, FREE], F32, isOutput=True)

    xb = [nc.alloc_sbuf_tensor(f"xb{i}", [128, F], F32).ap() for i in range(2)]
    tp = [nc.alloc_sbuf_tensor(f"tp{i}", [128, F], F32).ap() for i in range(2)]
    vr = [nc.alloc_sbuf_tensor(f"vr{i}", [128, F], F32).ap() for i in range(RING_V)]
    pp = [nc.alloc_sbuf_tensor(f"pp{i}", [128, F], F32).ap() for i in range(RING_P)]
    da = [nc.alloc_sbuf_tensor(f"da{i}", [128, F], F32).ap() for i in range(2)]
    dvo = [nc.alloc_sbuf_tensor(f"dvo{i}", [128, F], F32).ap() for i in range(2)]
    ga0 = [nc.alloc_sbuf_tensor(f"ga0{i}", [128, F], F32).ap() for i in range(2)]
    mb = [nc.alloc_sbuf_tensor(f"mb{i}", [128, F], F32).ap() for i in range(2)]
    yb = [nc.alloc_sbuf_tensor(f"yb{i}", [128, F], F32).ap() for i in range(2)]
    pb = nc.alloc_sbuf_tensor("pb", [128, 64], F32).ap()

    a_s = pb[:, 0:1]

    def gcol(i):               # gamma_i, i in 1..49
        return pb[:, 2 + i:3 + i]

    def unit_slice(u):
        g, ci = divmod(u, FREE // F)
        return g, ci * F

    with (nc.Block() as block,
          nc.semaphore("s_in") as s_in,
          nc.semaphore("s_act") as s_act,
          nc.semaphore("s_dve") as s_dve,
          nc.semaphore("s_gp") as s_gp,
          nc.semaphore("s_out") as s_out):

        @block.sync
        def _(sync):
            sync.dma_start(out=pb[:], in_=prm[:]).then_inc(s_in, 16)

            def dma_in(u):
                g, off = unit_slice(u)
                sync.dma_start(out=xb[u % 2][:],
                               in_=x_in[g, :, off:off + F]).then_inc(s_in, 16)

            dma_in(0)
            dma_in(1)
            for u in range(UNITS):
                sync.wait_ge(s_gp, N_GP * (u + 1))
                g, off = unit_slice(u)
                sync.dma_start(out=y_out[g, :, off:off + F],
                               in_=yb[u % 2][:]).then_inc(s_out, 16)
                if u + 2 < UNITS:
                    sync.wait_ge(s_act, N_ACT * (u + 1))
                    dma_in(u + 2)

        @block.scalar
        def _(scalar):
            for u in range(UNITS):
                scalar.wait_ge(s_in, 32 + 16 * u)
                if u >= 2:
                    # tp[u%2] free once DVE chain of unit u-2 is done
                    scalar.wait_ge(s_dve, N_DVE * (u - 1))
                scalar.activation(out=tp[u % 2][:], in_=xb[u % 2][:],
                                  func=Ident, scale=a_s,
                                  bias=pb[:, 52:53]).then_inc(s_act, 1)
                if u >= 2:
                    # ga0[u%2] read by merge 1 of unit u-2
                    scalar.wait_ge(s_gp, N_GP * (u - 2) + 1)
                scalar.activation(out=ga0[u % 2][:], in_=xb[u % 2][:],
                                  func=Ident, scale=pb[:, 62:63],
                                  bias=pb[:, 63:64]).then_inc(s_act, 1)
                for k in range(N_PART):
                    gidx = N_PART * u + k
                    if gidx >= RING_V:
                        # slot free once DVE partial op for gidx-RING_V retired
                        u2, k2 = divmod(gidx - RING_V, N_PART)
                        scalar.wait_ge(s_dve, N_DVE * u2 + k2 + 1)
                    scalar.activation(out=vr[gidx % RING_V][:], in_=xb[u % 2][:],
                                      func=Ident, scale=a_s,
                                      bias=pb[:, 53 + k:54 + k]).then_inc(s_act, 1)

        @block.vector
        def _(vector):
            for u in range(UNITS):
                # partials first so GP merges start early:
                # op k covers i = off..off+2 on stream v = t'-off
                for k, off in enumerate(P_OFFS):
                    gidx = N_PART * u + k
                    vector.wait_ge(s_act, N_ACT * u + 3 + k)   # v ready
                    if k == 0:
                        # p_0 seeds the chain; da[1] is free until chain op 1
                        out_p = da[1][:]
                    else:
                        hidx = (N_PART - 1) * u + (k - 1)
                        if hidx >= RING_P:
                            # pp slot free once GP merge for hidx-RING_P retired
                            u2, k2 = divmod(hidx - RING_P, N_PART - 1)
                            vector.wait_ge(s_gp, N_GP * u2 + k2 + 1)
                        out_p = pp[hidx % RING_P][:]
                    vector._custom_dve(PART, out=out_p,
                                       in0=vr[gidx % RING_V][:],
                                       in1=gcol(off + 2),
                                       s0=gcol(off),
                                       s1=gcol(off + 1)).then_inc(s_dve, 1)
                # chain: op 0 covers i=1,2 and absorbs partial p_0 (in da[1])
                # as its incoming accumulator; 10 pairs cover i=3..22
                tpa = tp[u % 2][:]
                vector._custom_dve(CHAIN, out=da[0][:], in0=tpa,
                                   in1=da[1][:],
                                   s0=gcol(1), s1=gcol(2),
                                   imm2=1.0).then_inc(s_dve, 1)
                for c in range(1, N_CHAIN):
                    j = 2 * c + 1
                    if c == N_CHAIN - 1:
                        out_ap = dvo[u % 2][:]
                        if u >= 2:
                            # dvo[u%2] read by final merge of unit u-2
                            vector.wait_ge(s_gp, N_GP * (u - 1))
                    else:
                        out_ap = da[c % 2][:]
                    vector._custom_dve(CHAIN, out=out_ap, in0=tpa,
                                       in1=da[(c - 1) % 2][:],
                                       s0=gcol(j), s1=gcol(j + 1),
                                       imm2=float(j)).then_inc(s_dve, 1)

        @block.gpsimd
        def _(gp):
            for u in range(UNITS):
                # merge seed + 9 partials, then + DVE chain
                for m in range(N_PART - 1):
                    hidx = (N_PART - 1) * u + m
                    gp.wait_ge(s_dve, N_DVE * u + m + 2)            # p_{m+1} done
                    if m == 0:
                        gp.wait_ge(s_act, N_ACT * u + 2)            # seed done
                        src = ga0[u % 2][:]
                    else:
                        src = mb[(m - 1) % 2][:]
                    gp.tensor_tensor(out=mb[m % 2][:], in0=src,
                                     in1=pp[hidx % RING_P][:],
                                     op=Alu.add).then_inc(s_gp, 1)
                gp.wait_ge(s_dve, N_DVE * (u + 1))                  # dvo done
                if u >= 2:
                    gp.wait_ge(s_out, 16 * (u - 1))                 # yb flushed
                gp.tensor_tensor(out=yb[u % 2][:],
                                 in0=mb[(N_PART - 2) % 2][:],
                                 in1=dvo[u % 2][:],
                                 op=Alu.add).then_inc(s_gp, 1)

    lower_extended_insts(nc)
    _built["nc"] = nc
    return nc


def kernel(x, coefficients_vect, scaling_coeffs_vect):
    from concourse.bass_utils import run_bass_kernel_spmd
    from concourse import bass2jax
    bass2jax.install_neuronx_cc_hook()

    x = np.ascontiguousarray(np.asarray(x, _f32))
    coeff = np.asarray(coefficients_vect, _f32).reshape(-1)
    scal = np.asarray(scaling_coeffs_vect, _f32).reshape(-1)

    prm_ch = _host_params(coeff, scal)                 # [64, 62]
    prm_full = np.ascontiguousarray(np.tile(prm_ch, (2, 1)))  # [128, 62]

    nb = N_BATCH // NCORES                             # 4 batches per core
    in_maps = []
    for i in range(NCORES):
        xi = x[nb * i:nb * (i + 1)].reshape(GROUPS, 128, FREE)
        in_maps.append({"x": np.ascontiguousarray(xi), "prm": prm_full})

    nc = _build()
    res = run_bass_kernel_spmd(nc, in_maps, list(range(NCORES)))

    out = np.empty((N_BATCH, N_CH, H, W), _f32)
    for i in range(NCORES):
        out[nb * i:nb * (i + 1)] = np.asarray(res.results[i]["y"]).reshape(
            nb, N_CH, H, W)
    return out
